# revision 4
# baseline (speedup 1.0000x reference)
# Trainium2 Bass kernel for nn_DifferentiableRAM (DRAW-style glimpse read).
#
# Per-core computation (pure data parallel over batch, 4 images/core):
#   p = flatten(X) @ W_loc + b_loc           (fp32 on PE, diag-bundle scheme)
#   gx,gy,s2,delta,gamma = transforms(p)     (DVE/ACT scalar stage)
#   FxT/FyT Gaussian filterbanks, normalized (DVE/ACT + ones-matmul)
#   out = gamma * Fy @ X @ FxT               (bf16 on PE, fp32 accumulate)
#
# Locnet scheme: for each of 12 row-chunks (rc) and 32 w-blocks of 16:
#   stationary = W_loc chunk [128, 80]  (16 w' x 5 j, natural layout)
#   moving     = X bundle   [128, 16, 4]  (16 w x 4 b via transposed AP)
#   accumulate into one PSUM [80, 64]; p[b,j] = sum_w psum[w*5+j, w*4+b]
# extracted via a DMA flatten to partition 0 + strided free-dim folds
# (compute engines require 32-aligned partition bases, hence no partition
# folds below 32).
import numpy as np

B, C, H, W = 32, 3, 512, 512
NB = 4
RC = 12
OUT = 64
FAN_IN = C * H * W
LN_DELTA = float(np.log(511.0 / 63.0))

_CACHE = {}


def _build():
    import concourse.bass as bass
    import concourse.mybir as mybir
    import concourse.tile as tile
    from concourse import bacc

    f32 = mybir.dt.float32
    bf16 = mybir.dt.bfloat16
    Exp = mybir.ActivationFunctionType.Exp
    sub_ = mybir.AluOpType.subtract
    mult_ = mybir.AluOpType.mult
    add_ = mybir.AluOpType.add

    nc = bacc.Bacc("TRN2", target_bir_lowering=False, debug=False)
    X = nc.dram_tensor("X", [NB, C, H, W], f32, kind="ExternalInput").ap()
    Wl = nc.dram_tensor("Wl", [FAN_IN, 5], f32, kind="ExternalInput").ap()
    bl = nc.dram_tensor("bl", [5], f32, kind="ExternalInput").ap()
    # aux_a[r, ch] = 128*ch + r ; aux_i[i] = i - 32.5
    aux_a = nc.dram_tensor("aux_a", [128, 4], f32, kind="ExternalInput").ap()
    aux_i = nc.dram_tensor("aux_i", [64], f32, kind="ExternalInput").ap()
    out = nc.dram_tensor("out", [NB, C, OUT, OUT], f32, kind="ExternalOutput").ap()

    Xv = X.rearrange("b c (hc p) w -> b (c hc) p w", p=128)  # [4, 12, 128, 512]
    Wv = Wl.rearrange("(rc p w) j -> rc p (w j)", rc=RC, p=128)  # [12, 128, 2560]

    with tile.TileContext(nc) as tc:
        with (
            tc.tile_pool(name="consts", bufs=1) as consts,
            tc.tile_pool(name="xfp", bufs=3) as xfp,
            tc.tile_pool(name="wtp", bufs=3) as wtp,
            tc.tile_pool(name="big", bufs=1) as big,
            tc.tile_pool(name="small", bufs=1) as small,
            tc.tile_pool(name="work", bufs=2) as work,
            tc.tile_pool(name="pls", bufs=1, space="PSUM") as pls,
            tc.tile_pool(name="pss", bufs=2, space="PSUM") as pss,
            tc.tile_pool(name="pgl", bufs=2, space="PSUM") as pgl,
        ):
            # ---- constants
            aux4 = consts.tile([128, 4], f32)
            nc.sync.dma_start(aux4[:], aux_a)
            auxi = consts.tile([1, 64], f32)
            nc.sync.dma_start(auxi[:], aux_i.rearrange("(o i) -> o i", o=1))
            ones = consts.tile([128, 1], f32)
            nc.vector.memset(ones[:], 1.0)
            bls = consts.tile([1, 5], f32)
            nc.sync.dma_start(bls[:], bl.rearrange("(o j) -> o j", o=1))

            # ---- X-resident bf16 copy for the glimpse
            xbf = big.tile([128, NB, RC, 512], bf16)

            # ---- locnet: stream W and X per row-chunk, accumulate PSUM
            psl = pls.tile([80, 64], f32)
            for rc in range(RC):
                wt = wtp.tile([128, 2560], f32, tag="wt")
                nc.sync.dma_start(wt[:], Wv[rc])
                xf = xfp.tile([128, NB, 512], f32, tag="xf")
                nc.sync.dma_start(xf[:], Xv[:, rc].rearrange("b p w -> p b w"))
                for b in range(NB):
                    nc.vector.tensor_copy(xbf[:, b, rc, :], xf[:, b, :])
                for wb in range(32):
                    nc.tensor.matmul(
                        psl[:],
                        wt[:, wb * 80 : wb * 80 + 80],
                        xf[:, :, wb * 16 : wb * 16 + 16].transpose([0, 2, 1]),
                        start=(rc == 0 and wb == 0),
                        stop=(rc == RC - 1 and wb == 31),
                    )

            # ---- extract p[b, j] = sum_w psl[w*5+j, w*4+b]
            ploc = small.tile([80, 64], f32)
            nc.vector.tensor_copy(ploc[:], psl[:])
            flat = small.tile([1, 5120], f32)
            nc.sync.dma_start(
                flat[0:1, :].rearrange("o (a x) -> o a x", a=80), ploc[:]
            )
            # diag at flat[0, w*324 + j*64 + b]; fold w 16->8->4->2->1
            for s in (8, 4, 2, 1):
                lo = bass.AP(
                    tensor=flat.tensor, offset=flat.offset,
                    ap=[[5120, 1], [324, s], [64, 5], [1, 4]],
                )
                hi = bass.AP(
                    tensor=flat.tensor, offset=flat.offset + 324 * s,
                    ap=[[5120, 1], [324, s], [64, 5], [1, 4]],
                )
                nc.vector.tensor_add(out=lo, in0=lo, in1=hi)
            # p[b, j] now at flat[0, j*64 + b]; add b_loc
            pv = bass.AP(
                tensor=flat.tensor, offset=flat.offset,
                ap=[[5120, 1], [64, 5], [1, 4]],
            )
            nc.vector.tensor_add(
                out=pv, in0=pv, in1=bls[0:1, :, None].to_broadcast((1, 5, 4))
            )

            def pslice(j, b):
                return flat[0:1, j * 64 + b : j * 64 + b + 1]

            # ---- per-b scalars and filters
            fxts = []
            fyts = []
            for b in range(NB):
                # row layout: [0:64] mx | [64:128] my | [128] ninv
                row = small.tile([1, 132], f32, name=f"row{b}")
                gxb = small.tile([1, 1], f32, name=f"gx{b}")
                nc.vector.tensor_scalar(
                    out=gxb[:], in0=pslice(0, b), scalar1=32.0, scalar2=32.0,
                    op0=mult_, op1=add_,
                )
                gyb = small.tile([1, 1], f32, name=f"gy{b}")
                nc.vector.tensor_scalar(
                    out=gyb[:], in0=pslice(1, b), scalar1=32.0, scalar2=32.0,
                    op0=mult_, op1=add_,
                )
                sigb = small.tile([1, 1], f32, name=f"sig{b}")
                nc.scalar.activation(sigb[:], pslice(2, b), Exp)
                nc.vector.reciprocal(row[0:1, 128:129], sigb[:])
                nc.vector.tensor_scalar_mul(row[0:1, 128:129], row[0:1, 128:129], -0.5)
                delb = small.tile([1, 1], f32, name=f"del{b}")
                nc.scalar.activation(delb[:], pslice(3, b), Exp)
                nc.vector.tensor_scalar_mul(delb[:], delb[:], 511.0 / 63.0)
                gamb = small.tile([1, 1], f32, name=f"gam{b}")
                nc.scalar.activation(gamb[:], pslice(4, b), Exp)
                # mx | my rows
                nc.vector.tensor_scalar(
                    out=row[0:1, 0:64], in0=auxi[:], scalar1=delb[0:1, 0:1],
                    scalar2=gxb[0:1, 0:1], op0=mult_, op1=add_,
                )
                nc.vector.tensor_scalar(
                    out=row[0:1, 64:128], in0=auxi[:], scalar1=delb[0:1, 0:1],
                    scalar2=gyb[0:1, 0:1], op0=mult_, op1=add_,
                )
                rbc = work.tile([128, 132], f32, tag="rbc")
                nc.gpsimd.partition_broadcast(rbc[:], row[0:1, :])

                for fi, (off, withg) in enumerate([(64, False), (0, True)]):
                    # fi=0: Fy (my), fi=1: Fx (mx, carries gamma)
                    ft = work.tile([128, 4, 64], f32, tag="ft")
                    nc.vector.tensor_tensor(
                        out=ft[:],
                        in0=rbc[:, off : off + 64][:, None, :].to_broadcast(
                            (128, 4, 64)
                        ),
                        in1=aux4[:, :, None].to_broadcast((128, 4, 64)),
                        op=sub_,
                    )
                    nc.vector.tensor_mul(ft[:], ft[:], ft[:])
                    nc.scalar.activation(
                        ft[:], ft[:], Exp, scale=rbc[:, 128:129]
                    )
                    pssum = pss.tile([1, 256], f32, tag="pssum")
                    nc.tensor.matmul(
                        pssum[:], ones[:, 0:1], ft[:], start=True, stop=True
                    )
                    srow = small.tile([1, 256], f32, name=f"srow{b}_{fi}")
                    nc.vector.tensor_copy(srow[:], pssum[:])
                    nc.vector.tensor_add(
                        out=srow[0:1, 0:128], in0=srow[0:1, 0:128],
                        in1=srow[0:1, 128:256],
                    )
                    nc.vector.tensor_add(
                        out=srow[0:1, 0:64], in0=srow[0:1, 0:64],
                        in1=srow[0:1, 64:128],
                    )
                    nc.vector.tensor_scalar_add(srow[0:1, 0:64], srow[0:1, 0:64], 1e-4)
                    nc.vector.reciprocal(srow[0:1, 0:64], srow[0:1, 0:64])
                    if withg:
                        nc.vector.tensor_scalar_mul(
                            srow[0:1, 0:64], srow[0:1, 0:64], gamb[0:1, 0:1]
                        )
                    nbc = work.tile([128, 64], f32, tag="nbc")
                    nc.gpsimd.partition_broadcast(nbc[:], srow[0:1, 0:64])
                    fbf = big.tile([128, 4, 64], bf16, name=f"fbf{b}_{fi}")
                    nc.vector.tensor_tensor(
                        out=fbf[:],
                        in0=ft[:],
                        in1=nbc[:, None, :].to_broadcast((128, 4, 64)),
                        op=mult_,
                    )
                    (fyts if fi == 0 else fxts).append(fbf)

            # ---- glimpse
            for b in range(NB):
                osb = small.tile([64, C, 64], f32, name=f"osb{b}")
                for c in range(C):
                    psfyx = pgl.tile([128, 4, 64], f32, tag="fyx")
                    for wb in range(4):
                        for hc in range(4):
                            rc = c * 4 + hc
                            nc.tensor.matmul(
                                psfyx[:, wb, :],
                                xbf[:, b, rc, wb * 128 : wb * 128 + 128],
                                fyts[b][:, hc, :],
                                start=(hc == 0),
                                stop=(hc == 3),
                            )
                    fyxbf = work.tile([128, 4, 64], bf16, tag="fyxbf")
                    nc.vector.tensor_copy(fyxbf[:], psfyx[:])
                    psout = pgl.tile([64, 64], f32, tag="out")
                    for wb in range(4):
                        nc.tensor.matmul(
                            psout[:],
                            fyxbf[:, wb, :],
                            fxts[b][:, wb, :],
                            start=(wb == 0),
                            stop=(wb == 3),
                        )
                    nc.vector.tensor_copy(osb[:, c, :], psout[:])
                nc.sync.dma_start(out[b].rearrange("c n m -> n c m"), osb[:])

    nc.compile()
    return nc


def _aux_inputs():
    aux_a = (
        np.arange(128, dtype=np.float32)[:, None]
        + 128.0 * np.arange(4, dtype=np.float32)[None, :]
    )
    aux_i = np.arange(64, dtype=np.float32) - 32.5
    return np.ascontiguousarray(aux_a), np.ascontiguousarray(aux_i)


def kernel(X, W_loc, b_loc):
    from concourse.bass_utils import run_bass_kernel_spmd

    X = np.ascontiguousarray(np.asarray(X, dtype=np.float32))
    W_loc = np.ascontiguousarray(np.asarray(W_loc, dtype=np.float32))
    b_loc = np.ascontiguousarray(np.asarray(b_loc, dtype=np.float32))

    if "nc" not in _CACHE:
        _CACHE["nc"] = _build()
    nc = _CACHE["nc"]

    aux_a, aux_i = _aux_inputs()
    n_cores = 8
    in_maps = [
        {
            "X": np.ascontiguousarray(X[c * NB : (c + 1) * NB]),
            "Wl": W_loc,
            "bl": b_loc,
            "aux_a": aux_a,
            "aux_i": aux_i,
        }
        for c in range(n_cores)
    ]
    res = run_bass_kernel_spmd(nc, in_maps, core_ids=list(range(n_cores)))
    return np.concatenate([r["out"] for r in res.results], axis=0)


# revision 16
# speedup vs baseline: 1.1082x; 1.1082x over previous
# Trainium2 Bass kernel for nn_DifferentiableRAM (DRAW-style glimpse read).
#
# Per-core computation (pure data parallel over batch, 4 images/core):
#   p = flatten(X) @ W_loc + b_loc           (fp32 on PE, diag-bundle scheme)
#   gx,gy,s2,delta,gamma = transforms(p)     (DVE/ACT scalar stage)
#   FxT/FyT Gaussian filterbanks              (DVE sub/sq, ACT exp -> bf16)
#   out = gamma * Fy @ X @ FxT               (bf16 on PE, fp32 accumulate)
#
# Locnet scheme: for each of 24 half-row-chunks (rc, h) and 16 w-blocks:
#   stationary = W_loc chunk [128, 80]  (16 w' x 5 j, natural layout)
#   moving     = X bundle   [128, 16, 4]  (16 w x 4 b via transposed AP)
#   accumulate into one PSUM [80, 64]; p[b,j] = sum_w psum[w*5+j, w*4+b]
# extracted via a DMA flatten to partition 0 + a strided tensor_reduce
# (compute engines require 32-aligned partition bases, hence no partition
# folds below 32).
#
# Normalization: 1/(Sy+eps) is multiplied into FyT before stage 1;
# gamma/(Sx+eps) is fused into the final PSUM->SBUF output copy along m.
# Filter sums come from accumulating ones-matmuls over the bf16 filters.
import numpy as np

B, C, H, W = 32, 3, 512, 512
NB = 4
RC = 12
OUT = 64
FAN_IN = C * H * W

_CACHE = {}


def _build(skip_locnet=False, skip_glimpse=False, skip_cast=False):
    import concourse.bass as bass
    import concourse.mybir as mybir
    import concourse.tile as tile
    from concourse import bacc

    f32 = mybir.dt.float32
    bf16 = mybir.dt.bfloat16
    Exp = mybir.ActivationFunctionType.Exp
    sub_ = mybir.AluOpType.subtract
    mult_ = mybir.AluOpType.mult
    add_ = mybir.AluOpType.add

    nc = bacc.Bacc("TRN2", target_bir_lowering=False, debug=False)
    X = nc.dram_tensor("X", [NB, C, H, W], f32, kind="ExternalInput").ap()
    Wl = nc.dram_tensor("Wl", [FAN_IN, 5], f32, kind="ExternalInput").ap()
    bl = nc.dram_tensor("bl", [5], f32, kind="ExternalInput").ap()
    # aux_a[r, ch] = -(128*ch + r) ; aux_i[i] = i - 32.5
    aux_a = nc.dram_tensor("aux_a", [128, 4], f32, kind="ExternalInput").ap()
    aux_i = nc.dram_tensor("aux_i", [64], f32, kind="ExternalInput").ap()
    out = nc.dram_tensor("out", [NB, C, OUT, OUT], f32, kind="ExternalOutput").ap()

    Xv = X.rearrange("b c (hc p) (wh w) -> b (c hc) wh p w", p=128, wh=2)
    # [4, 12, 2, 128, 256]
    Wv = Wl.rearrange("(rc p wh w) j -> rc wh p (w j)", rc=RC, p=128, wh=2)
    # [12, 2, 128, 1280]

    with tile.TileContext(nc) as tc:
        with (
            tc.tile_pool(name="consts", bufs=1) as consts,
            tc.tile_pool(name="xfp", bufs=6) as xfp,
            tc.tile_pool(name="wtp", bufs=6) as wtp,
            tc.tile_pool(name="big", bufs=1) as big,
            tc.tile_pool(name="small", bufs=1) as small,
            tc.tile_pool(name="work", bufs=4) as work,
            tc.tile_pool(name="pls", bufs=1, space="PSUM") as pls,
            tc.tile_pool(name="pss", bufs=3, space="PSUM") as pss,
            tc.tile_pool(name="pgl", bufs=2, space="PSUM") as pgl,
        ):
            aux4 = consts.tile([128, 4], f32)
            auxi = consts.tile([1, 64], f32)
            ones = consts.tile([128, 1], bf16)
            bls = consts.tile([1, 5], f32)
            xbf = big.tile([128, NB, RC, 512], bf16)

            # ---- locnet: stream W and X per half-row-chunk, accumulate PSUM
            psl = pls.tile([80, 64], f32)
            for rc in range(RC):
                for wh in range(2):
                    wt = wtp.tile([128, 1280], f32, tag="wt")
                    nc.sync.dma_start(wt[:], Wv[rc, wh])
                    xf = xfp.tile([128, NB, 256], f32, tag="xf")
                    nc.sync.dma_start(
                        xf[:], Xv[:, rc, wh].rearrange("b p w -> p b w")
                    )
                    if rc == 0 and wh == 0:
                        # const loads off the HWDGE path (SWDGE via gpsimd)
                        nc.gpsimd.dma_start(aux4[:], aux_a)
                        nc.gpsimd.dma_start(
                            auxi[:], aux_i.rearrange("(o i) -> o i", o=1)
                        )
                        nc.gpsimd.dma_start(
                            bls[:], bl.rearrange("(o j) -> o j", o=1)
                        )
                        nc.vector.memset(ones[:], 1.0)
                    if not skip_cast:
                        for b in range(NB):
                            nc.vector.tensor_copy(
                                xbf[:, b, rc, wh * 256 : wh * 256 + 256],
                                xf[:, b, :],
                            )
                    if skip_locnet:
                        if rc == 0 and wh == 0:
                            nc.tensor.matmul(
                                psl[:], wt[:, 0:80],
                                xf[:, :, 0:16].transpose([0, 2, 1]),
                                start=True, stop=True,
                            )
                        continue
                    for wb in range(16):
                        nc.tensor.matmul(
                            psl[:],
                            wt[:, wb * 80 : wb * 80 + 80],
                            xf[:, :, wb * 16 : wb * 16 + 16].transpose([0, 2, 1]),
                            start=(rc == 0 and wh == 0 and wb == 0),
                            stop=(rc == RC - 1 and wh == 1 and wb == 15),
                        )

            # ---- extract p[b, j] = sum_w psl[w*5+j, w*4+b]
            ploc = small.tile([80, 64], f32)
            nc.vector.tensor_copy(ploc[:], psl[:])
            flat = small.tile([1, 5120], f32)
            nc.sync.dma_start(
                flat[0:1, :].rearrange("o (a x) -> o a x", a=80), ploc[:]
            )
            # diag at flat[0, w*324 + j*64 + b]; reduce over w in one op
            diag = bass.AP(
                tensor=flat.tensor, offset=flat.offset,
                ap=[[5120, 1], [64, 5], [1, 4], [324, 16]],
            )
            pfin = small.tile([1, 5, 4], f32)
            nc.vector.tensor_reduce(
                out=pfin[0:1], in_=diag, axis=mybir.AxisListType.X,
                op=mybir.AluOpType.add,
            )
            nc.vector.tensor_add(
                out=pfin[0:1], in0=pfin[0:1],
                in1=bls[0:1, :, None].to_broadcast((1, 5, 4)),
            )

            def pslice(j, b):
                return pfin[0:1, j, b : b + 1]

            # ---- scalar stage (batched across b where possible)
            # g2 = 32*p[0:2] + 32  (gx row j=0, gy row j=1)
            g2 = small.tile([1, 2, 4], f32)
            nc.vector.tensor_scalar(
                out=g2[0:1], in0=pfin[0:1, 0:2, :], scalar1=32.0, scalar2=32.0,
                op0=mult_, op1=add_,
            )
            # sdg = exp(p[2:5]): rows = sigma2 | delta/8.111 | gamma
            sdg = small.tile([1, 3, 4], f32)
            nc.scalar.activation(sdg[0:1], pfin[0:1, 2:5, :], Exp)
            # delta row *= 511/63
            nc.vector.tensor_scalar_mul(
                sdg[0:1, 1, :], sdg[0:1, 1, :], 511.0 / 63.0
            )
            # ninv row = -1/(2*sigma2)
            nrow = small.tile([1, 4], f32)
            nc.vector.tensor_scalar_mul(nrow[0:1, :], sdg[0:1, 0, :], -2.0)
            nc.vector.reciprocal(nrow[0:1, :], nrow[0:1, :])
            gams = [sdg[0:1, 2, b : b + 1] for b in range(NB)]

            rows = []
            for b in range(NB):
                # row layout: [0:64] mx | [64:128] my | [128] ninv
                row = small.tile([1, 132], f32, name=f"row{b}")
                nc.vector.tensor_scalar(
                    out=row[0:1, 0:64], in0=auxi[:],
                    scalar1=sdg[0:1, 1, b : b + 1],
                    scalar2=g2[0:1, 0, b : b + 1], op0=mult_, op1=add_,
                )
                nc.vector.tensor_scalar(
                    out=row[0:1, 64:128], in0=auxi[:],
                    scalar1=sdg[0:1, 1, b : b + 1],
                    scalar2=g2[0:1, 1, b : b + 1], op0=mult_, op1=add_,
                )
                nc.vector.tensor_copy(row[0:1, 128:129], nrow[0:1, b : b + 1])
                rows.append(row)

            # ---- filters + glimpse, per-b so b's glimpse overlaps (b+1)'s
            # filter build.  Within b the two filter chains run stage-wise.
            rbcs = []
            for b in range(NB):
                rbc = work.tile([128, 132], f32, tag="rbc")
                nc.gpsimd.partition_broadcast(rbc[:], rows[b][0:1, :])
                rbcs.append(rbc)
            Square = mybir.ActivationFunctionType.Square
            for b in range(NB if not skip_glimpse else 0):
                fts, fbfs, pssums, srows, nbcs = {}, {}, {}, {}, {}
                for fi, off in ((0, 64), (1, 0)):  # fi=0: Fy(my); 1: Fx(mx)
                    # d^2 = Square(m*1 + (-a)) per chunk, entirely on ACT
                    # (aux4 holds NEGATED a values)
                    ft = work.tile(
                        [128, 4, 64], f32, tag=f"ft{fi}", name=f"ft{b}_{fi}",
                        bufs=2,
                    )
                    for c4 in range(4):
                        nc.scalar.activation(
                            ft[:, c4, :],
                            rbcs[b][:, off : off + 64],
                            Square,
                            bias=aux4[:, c4 : c4 + 1],
                        )
                    fts[fi] = ft
                for fi in (0, 1):
                    fbf = big.tile([128, 4, 64], bf16, name=f"fbf{b}_{fi}")
                    nc.scalar.activation(
                        fbf[:], fts[fi][:], Exp, scale=rbcs[b][:, 128:129]
                    )
                    fbfs[fi] = fbf
                for fi in (0, 1):
                    pssum = pss.tile([1, 64], f32, tag="pssum")
                    for c4 in range(4):
                        nc.tensor.matmul(
                            pssum[:], ones[:, 0:1], fbfs[fi][:, c4, :],
                            start=(c4 == 0), stop=(c4 == 3),
                        )
                    pssums[fi] = pssum
                for fi in (0, 1):
                    srow = small.tile([1, 64], f32, name=f"srow{b}_{fi}")
                    nc.vector.tensor_scalar_add(
                        srow[0:1, :], pssums[fi][:], 1e-4
                    )
                    nc.vector.reciprocal(srow[0:1, :], srow[0:1, :])
                    if fi == 1:
                        nc.vector.tensor_scalar_mul(
                            srow[0:1, :], srow[0:1, :], gams[b]
                        )
                    srows[fi] = srow
                for fi in (0, 1):
                    nbc = work.tile(
                        [128, 64], f32, tag=f"nbc{fi}", name=f"nbc{b}_{fi}",
                        bufs=2,
                    )
                    nc.gpsimd.partition_broadcast(nbc[:], srows[fi][0:1, :])
                    nbcs[fi] = nbc
                # normalize FyT (bf16 * f32-bcast -> bf16)
                fyn = big.tile([128, 4, 64], bf16, name=f"fyn{b}")
                nc.vector.tensor_tensor(
                    out=fyn[:],
                    in0=fbfs[0][:],
                    in1=nbcs[0][:, None, :].to_broadcast((128, 4, 64)),
                    op=mult_,
                )

                # glimpse for this b
                osb = small.tile([64, C, 64], f32, name=f"osb{b}")
                psfyxs, fyxbfs, psouts = {}, {}, {}
                for c in range(C):
                    psfyx = pgl.tile([128, 4, 64], f32, tag="fyx")
                    for wb in range(4):
                        for hc in range(4):
                            rc = c * 4 + hc
                            nc.tensor.matmul(
                                psfyx[:, wb, :],
                                xbf[:, b, rc, wb * 128 : wb * 128 + 128],
                                fyn[:, hc, :],
                                start=(hc == 0),
                                stop=(hc == 3),
                            )
                    psfyxs[c] = psfyx
                for c in range(C):
                    fyxbf = work.tile([128, 4, 64], bf16, tag="fyxbf")
                    nc.vector.tensor_copy(fyxbf[:], psfyxs[c][:])
                    fyxbfs[c] = fyxbf
                for c in range(C):
                    psout = pgl.tile([64, 64], f32, tag="out")
                    for wb in range(4):
                        nc.tensor.matmul(
                            psout[:],
                            fyxbfs[c][:, wb, :],
                            fbfs[1][:, wb, :],
                            start=(wb == 0),
                            stop=(wb == 3),
                        )
                    psouts[c] = psout
                for c in range(C):
                    # scale by gamma/(Sx+eps) along m while leaving PSUM
                    nc.vector.tensor_mul(
                        osb[:, c, :], psouts[c][:], nbcs[1][0:64, :]
                    )
                nc.sync.dma_start(
                    out[b].rearrange("c n m -> n c m"), osb[:]
                )

    nc.compile()
    return nc


def _aux_inputs():
    aux_a = -(
        np.arange(128, dtype=np.float32)[:, None]
        + 128.0 * np.arange(4, dtype=np.float32)[None, :]
    )
    aux_i = np.arange(64, dtype=np.float32) - 32.5
    return np.ascontiguousarray(aux_a), np.ascontiguousarray(aux_i)


def kernel(X, W_loc, b_loc):
    from concourse.bass_utils import run_bass_kernel_spmd

    X = np.ascontiguousarray(np.asarray(X, dtype=np.float32))
    W_loc = np.ascontiguousarray(np.asarray(W_loc, dtype=np.float32))
    b_loc = np.ascontiguousarray(np.asarray(b_loc, dtype=np.float32))

    if "nc" not in _CACHE:
        _CACHE["nc"] = _build()
    nc = _CACHE["nc"]

    aux_a, aux_i = _aux_inputs()
    n_cores = 8
    in_maps = [
        {
            "X": np.ascontiguousarray(X[c * NB : (c + 1) * NB]),
            "Wl": W_loc,
            "bl": b_loc,
            "aux_a": aux_a,
            "aux_i": aux_i,
        }
        for c in range(n_cores)
    ]
    res = run_bass_kernel_spmd(nc, in_maps, core_ids=list(range(n_cores)))
    return np.concatenate([r["out"] for r in res.results], axis=0)
